# revision 20
# baseline (speedup 1.0000x reference)
"""AttVlad Trainium2 kernel.

Math (per image n):
  xn = x / ||x||_2(over d)                       x: [D=128, S]
  a  = softmax_k(conv_w @ xn + conv_b)           a: [K=64, S]
  vlad[k,d]   = sum_s a[k,s] xn[d,s] - (sum_s a[k,s]) centroids[k,d]
  out = normalize_d(vlad * (centroids @ att_w.T + att_b))

Device strategy (8 cores, data-parallel over n, 4 images each), per
128-position unit u of each 4096-position chunk:
  - x is cast to bf16 on the host (device math is bf16 either way) and
    streamed in [128d, 4096s] chunks on the SP HWDGE queue.
  - PE per unit: (1) rank-1 fp8-DoubleRow seed writes norm[s]*b[k] into
    the logits PSUM, (2) bf16 logits matmul accumulates on top, so
    exp(rn*(l + norm*b)) = exp(rn*l + b) and the softmax bias multiply
    disappears; (3) an is_transpose matmul produces x^T in *bf16* PSUM.
  - lsc = psl*rn (DVE, broadcast); e = exp(lsc) (ACT, chunk-batched,
    k-major layout); dn = sum_k e (Pool/DVE); a2 = e * (rn*rdn) with the
    per-(s,u) factor materialized k-major by a 4x-mode tensor_copy so
    the a2 multiply runs in the DVE 2x bf16 mode.
  - x^T moves PSUM->SBUF as bf16 (DVE 2x tensor_copy / ACT copy / Pool,
    split per group) into a [128s, 130]-strided tile whose col 128 holds
    norm = sqrt(sumsq), so one 129-col VLAD matmul accumulates both
    A = sum_s a2 x^T and asum.
  - sumsq via natural-layout squares (ACT/DVE split) + per-unit ones
    matmul; rn = exp(-0.5 ln ss) stays in one ACT table set.
  - Host does the O(N*K*D) finalize (centroid subtract, attention scale,
    intra-normalization) in float64.
"""

import sys
import time

import numpy as np

try:  # the concourse stack (bass) ships in the container image
    import concourse.bass as _probe  # noqa: F401
except Exception:  # pragma: no cover
    sys.path.insert(0, "/opt/trn_rl_repo")

import ml_dtypes

N, D, S, K = 32, 128, 16384, 64
NCORES = 8
EPS = 1e-12

CHUNK = 4096  # s-positions per DMA chunk
UNIT = 128    # s-positions per matmul unit (psum partition dim)
XT_STRIDE = 130  # x^T unit stride in the SBUF tile: 128 cols x^T + 1 norm + 1 pad
UPC = CHUNK // UNIT  # units per chunk (32)
HU = 8               # units per pst group

# engine-split tunables (sweepable via the cost model)
XT_PATTERN = ("act", "act", "act", "act")   # xtmove engine per group g
DN_ENGINE = "dve"                            # dn reduce engine
A2_ENGINE = "pool"                           # a2 = e * ccb engine
XSQ_DVE_COLS = 512                          # chunk cols squared on DVE (rest ACT)


# this walrus build rejects >1 sync wait on every instruction struct probed
# (CTRL, TT, MM); keep both caps at 1
MAX_WAITS = 1
COMPUTE_WAITS = 1
_COMPUTE_TYPES = (
    "InstTensorTensor", "InstActivation", "InstMatmult", "InstTensorReduce",
    "InstReciprocal", "InstTensorCopy", "InstLdweights", "InstTensorScalarPtr",
    "InstMemSet", "InstTensorScalar",
)


def _split_waits(nc, mybir):
    """Rewrite the traced BIR so no instruction carries more sem waits than
    this walrus build's per-struct limit: excess waits move to injected NoOps
    immediately preceding the instruction on the same engine (NX executes
    waits in order, so this is semantically identical)."""
    nid = 0
    for f in nc.m.functions:
        for blk in f.blocks:
            new_insts = []
            for inst in blk.instructions:
                si = getattr(inst, "sync_info", None)
                ws = list(si.on_wait) if si is not None else []
                maxw = (
                    COMPUTE_WAITS
                    if type(inst).__name__ in _COMPUTE_TYPES
                    else MAX_WAITS
                )
                if len(ws) > maxw:
                    extra = ws[: len(ws) - maxw]
                    for i in range(0, len(extra), MAX_WAITS):
                        nid += 1
                        nop = mybir.InstNoOp(
                            name=f"waitsplit_{nid}", ins=[], outs=[]
                        )
                        nop.engine = inst.engine
                        nop.sync_info = mybir.SyncInfo(
                            on_wait=extra[i : i + MAX_WAITS], on_update=[]
                        )
                        new_insts.append(nop)
                    si.on_wait = ws[len(ws) - maxw :]
                new_insts.append(inst)
            blk.instructions[:] = new_insts


def build_program(n_per_core=4, s_total=S, reps=1, n_read=None):
    """Build the single-core Bass program (same program runs on all cores)."""
    if n_read is None:
        n_read = n_per_core
    import concourse.bass as bass
    import concourse.tile as tile
    from concourse import mybir

    dt = mybir.dt
    AF = mybir.ActivationFunctionType
    OP = mybir.AluOpType
    PM = mybir.MatmulPerfMode

    n_chunks = s_total // CHUNK

    nc = bass.Bass()
    x_in = nc.declare_dram_parameter(
        "x", [n_per_core, D, s_total], dt.bfloat16, isOutput=False
    )
    wt_in = nc.declare_dram_parameter("wt", [D, K], dt.bfloat16, isOutput=False)
    idm_in = nc.declare_dram_parameter("idm", [D, D], dt.bfloat16, isOutput=False)
    ones_in = nc.declare_dram_parameter("ones", [D, 1], dt.bfloat16, isOutput=False)
    bseed_in = nc.declare_dram_parameter(
        "bseed", [1, 2 * K], dt.float8e4, isOutput=False
    )
    out_dram = nc.declare_dram_parameter(
        "out", [K, n_per_core * 132], dt.float32, isOutput=True
    )

    with tile.TileContext(nc) as tc:
        with (
            tc.tile_pool(name="consts", bufs=1) as consts,
            tc.tile_pool(name="xc", bufs=4) as xc_pool,
            tc.tile_pool(name="xt", bufs=5) as xt_pool,
            tc.tile_pool(name="soft", bufs=4) as soft_pool,
            tc.tile_pool(name="stats", bufs=5) as stats_pool,
            tc.tile_pool(name="scratch", bufs=2) as scratch_pool,
            tc.tile_pool(name="seed", bufs=3) as seed_pool,
            tc.tile_pool(name="outp", bufs=1) as out_pool,
            tc.tile_pool(name="psl", bufs=2, space="PSUM") as psl_pool,
            tc.tile_pool(name="pv", bufs=2, space="PSUM") as pv_pool,
            tc.tile_pool(name="pss", bufs=2, space="PSUM") as pss_pool,
        ):
            wt = consts.tile([D, K], dt.bfloat16)
            nc.sync.dma_start(wt[:], wt_in[:])
            idm = consts.tile([D, D], dt.bfloat16)
            nc.sync.dma_start(idm[:], idm_in[:])
            ones = consts.tile([D, 1], dt.bfloat16)
            nc.sync.dma_start(ones[:], ones_in[:])
            bseed = consts.tile([1, 2 * K], dt.float8e4)
            nc.sync.dma_start(bseed[:], bseed_in[:])
            bseed3 = bseed[:].rearrange("p (j n) -> p j n", j=2)

            out_sb = out_pool.tile([K, n_per_core * 132], dt.float32)
            # touch the ln/exp ACT table set immediately so its ~2.7us DMA
            # overlaps the initial input loads instead of the first chunk
            warm = consts.tile([1, 1], dt.float32)
            nc.scalar.activation(warm[:], ones[0:1, 0:1], AF.Ln)

            def emit_all():
              chunk_list = [
                  (n, ci) for n in range(n_read) for ci in range(n_chunks)
              ]
              NL = len(chunk_list)
              HC = UPC // 2  # units per half-chunk stage (16)
              st = {}
              pv_state = {}

              def fetch(i):
                  """Issue the chunk load and the XBAR transpose (DMA)."""
                  n, ci = chunk_list[i]
                  xc = xc_pool.tile([D, CHUNK], dt.bfloat16, name="xc")
                  QC = CHUNK // 4
                  if i == 0:
                      for q in range(4):
                          nc.sync.dma_start(
                              xc[:, q * QC : (q + 1) * QC],
                              x_in[n, :, ci * CHUNK + q * QC
                                   : ci * CHUNK + (q + 1) * QC],
                          )
                  else:
                      nc.sync.dma_start(
                          xc[:], x_in[n, :, ci * CHUNK : (ci + 1) * CHUNK]
                      )
                  # x^T via the XBAR DMA transpose (u-major: xt3[p,u,:] =
                  # x^T[u*128+p]); rides the otherwise-idle DMA rails
                  xt = xt_pool.tile([128, UPC * UNIT], dt.bfloat16,
                                    name="xt")
                  xt3 = xt[:].rearrange("p (u c) -> p u c", c=UNIT)
                  nc.sync.dma_start_transpose(xt3, xc[:])
                  st[i] = {"xc": xc, "xt": xt}

              def prep_sq(i):
                  """Squares for chunk i (ACT, with a DVE share)."""
                  t = st[i]
                  xc = t["xc"]
                  xsq = scratch_pool.tile(
                      [D, CHUNK], dt.bfloat16, tag="xsq", name="xsq"
                  )
                  cd = XSQ_DVE_COLS
                  if i == 0:
                      QC = CHUNK // 4
                      for q in range(4):
                          nc.vector.tensor_tensor(
                              out=xsq[:, q * QC : (q + 1) * QC],
                              in0=xc[:, q * QC : (q + 1) * QC],
                              in1=xc[:, q * QC : (q + 1) * QC], op=OP.mult,
                          )
                  else:
                      if cd > 0:
                          nc.vector.tensor_tensor(
                              out=xsq[:, 0:cd], in0=xc[:, 0:cd],
                              in1=xc[:, 0:cd], op=OP.mult,
                          )
                      if cd < CHUNK:
                          nc.scalar.activation(
                              xsq[:, cd:CHUNK], xc[:, cd:CHUNK], AF.Square
                          )
                  t["xsq"] = xsq

              def prep_stats(i):
                  """sumsq matmuls, rn, norm, fp8 seed row for chunk i."""
                  t = st[i]
                  xsq = t.pop("xsq")
                  # pss bank: ss [128, 0:32] fp32; seedT [32, 32:96]->bf16
                  pss = pss_pool.tile([128, 96], dt.float32, name="pss")
                  ss = pss[:, 0:32]
                  for cu in range(UPC):
                      nc.tensor.matmul(
                          ss[:, cu : cu + 1],
                          xsq[:, cu * UNIT : (cu + 1) * UNIT],
                          ones[:], start=True, stop=True,
                      )
                  lns = stats_pool.tile([128, UPC], dt.float32, tag="lns",
                                        name="lns")
                  rn = stats_pool.tile([128, UPC], dt.float32, tag="rn",
                                       name="rn")
                  nc.scalar.activation(lns[:], ss, AF.Ln)
                  nc.scalar.activation(rn[:], lns[:], AF.Exp, scale=-0.5)
                  # norm = sqrt(ss), compact bf16 (seed row + asum column)
                  nr16 = stats_pool.tile([128, UPC], dt.bfloat16, tag="nr16",
                                         name="nr16")
                  nc.scalar.activation(nr16[:], lns[:], AF.Exp, scale=0.5)
                  # seed row: transpose norms to [32s-units, 128] then fp8
                  seedT = pss[0:32, 32:96].bitcast(dt.bfloat16)
                  nc.tensor.transpose(seedT, nr16[:], idm[:])
                  seed8 = seed_pool.tile([UPC, UNIT], dt.float8e4,
                                         tag="s8", name="seed8")
                  nc.scalar.activation(seed8[:], seedT, AF.Copy)
                  # partition-collapse: PE stationary base partition must be
                  # 0/32/64, so gather all unit rows onto partition 0
                  seed8f = seed_pool.tile([1, UPC * UNIT], dt.float8e4,
                                          tag="s8f", name="seed8f")
                  nc.sync.dma_start(seed8f[:], seed8[:])
                  t.update(rn=rn, nr16=nr16, seed8f=seed8f)

              def front_mm(i):
                  """Seed + logits matmuls for chunk i (both halves)."""
                  t = st[i]
                  xc, seed8f = t["xc"], t.pop("seed8f")
                  psls = []
                  for h in range(2):
                      psl = psl_pool.tile([128, HC * K], dt.float32,
                                          name="psl")
                      for ul in range(HC):
                          u = h * HC + ul
                          srow = seed8f[0:1, u * UNIT : (u + 1) * UNIT][
                              :, None, :
                          ]
                          nc.tensor.matmul(
                              psl[:, ul * K : (ul + 1) * K],
                              srow.broadcast_to([1, 2, UNIT]),
                              bseed3,
                              start=True, stop=False, perf_mode=PM.DoubleRow,
                          )
                          nc.tensor.matmul(
                              psl[:, ul * K : (ul + 1) * K],
                              xc[:, u * UNIT : (u + 1) * UNIT], wt[:],
                              start=False, stop=True,
                          )
                      psls.append(psl)
                  t["psls"] = psls

              def front_lsc(i, h):
                  """lsc_h = psl_h * rn_h, k-major [p, (k,64),(u,16)]."""
                  t = st[i]
                  psl = t["psls"][h]
                  rn = t["rn"]
                  lsc = soft_pool.tile([128, HC * K], dt.bfloat16, tag="lsc",
                                       name="lsc")
                  lsc_v = lsc[:].rearrange("p (k u) -> p k u", u=HC)
                  psl_v = psl[:].rearrange("p (u k) -> p k u", k=K)
                  rn_v = rn[:, h * HC : (h + 1) * HC][:, None, :]
                  nc.vector.tensor_tensor(
                      out=lsc_v, in0=psl_v,
                      in1=rn_v.broadcast_to([128, K, HC]), op=OP.mult,
                  )
                  t[("lsc", h)] = lsc

              def back_exp(i, h):
                  t = st[i]
                  lsc = t.pop(("lsc", h))
                  et = soft_pool.tile([128, HC * K], dt.bfloat16, tag="e",
                                      name="e")
                  nc.scalar.activation(et[:], lsc[:], AF.Exp)
                  t[("et", h)] = et

              def back_soft(i, h):
                  """dn_h + per-(s,u) scalar chain + ccb_h (DVE)."""
                  t = st[i]
                  et = t[("et", h)]
                  rn = t["rn"]
                  dn = stats_pool.tile([128, HC], dt.float32, tag="dn",
                                       name="dn")
                  e_ku = et[:].rearrange("p (k u) -> p u k", u=HC)
                  nc.vector.tensor_reduce(
                      out=dn[:], in_=e_ku, axis=mybir.AxisListType.X,
                      op=OP.add,
                  )
                  rdn = stats_pool.tile([128, HC], dt.float32, tag="rdn",
                                        name="rdn")
                  nc.vector.reciprocal(rdn[:], dn[:])
                  cc = stats_pool.tile([128, HC], dt.float32, tag="cc",
                                       name="cc")
                  nc.vector.tensor_tensor(
                      out=cc[:], in0=rn[:, h * HC : (h + 1) * HC],
                      in1=rdn[:], op=OP.mult,
                  )
                  cc16 = stats_pool.tile([128, HC], dt.bfloat16, tag="cc16",
                                         name="cc16")
                  nc.vector.tensor_copy(cc16[:], cc[:])
                  ccb = soft_pool.tile([128, HC * K], dt.bfloat16, tag="ccb",
                                       name="ccb")
                  nc.vector.tensor_copy(
                      ccb[:].rearrange("p (k u) -> p k u", u=HC),
                      cc16[:][:, None, :].broadcast_to([128, K, HC]),
                  )
                  t[("ccb", h)] = ccb

              def back_a2(i, h):
                  t = st[i]
                  et, ccb = t.pop(("et", h)), t.pop(("ccb", h))
                  a2 = soft_pool.tile([128, HC * K], dt.bfloat16, tag="a2",
                                      name="a2")
                  if A2_ENGINE == "pool":
                      nc.gpsimd.tensor_tensor(out=a2[:], in0=et[:],
                                              in1=ccb[:], op=OP.mult)
                  else:
                      nc.vector.tensor_tensor(out=a2[:], in0=et[:],
                                              in1=ccb[:], op=OP.mult)
                  t[("a2", h)] = a2

              def back_vlad(i):
                  """VLAD + asum accumulation; output on the last chunk."""
                  n, ci = chunk_list[i]
                  t = st.pop(i)
                  xt, nr16 = t["xt"], t["nr16"]
                  if ci == 0:
                      pv_state[n] = pv_pool.tile([K, 132], dt.float32,
                                                 name="pv")
                  pv = pv_state[n]
                  for h in range(2):
                      a2 = t[("a2", h)]
                      a2_ku = a2[:].rearrange("p (k u) -> p u k", u=HC)
                      for ul in range(HC):
                          u = h * HC + ul
                          cu = ci * UPC + u
                          first = cu == 0
                          last = cu == (s_total // UNIT) - 1
                          nc.tensor.matmul(
                              pv[:, 0:D],
                              a2_ku[:, ul, :],
                              xt[:, u * UNIT : (u + 1) * UNIT],
                              start=first, stop=last,
                          )
                          nc.tensor.matmul(
                              pv[:, D : D + 1],
                              a2_ku[:, ul, :],
                              nr16[:, u : u + 1],
                              start=first, stop=last,
                          )
                  if ci == n_chunks - 1:
                      nc.scalar.activation(
                          out_sb[:, n * 132 : n * 132 + D + 1],
                          pv[:, 0 : D + 1], AF.Copy,
                      )
                      nc.sync.dma_start(
                          out_dram[:, n * 132 : n * 132 + D + 1],
                          out_sb[:, n * 132 : n * 132 + D + 1],
                      )

              # Software pipeline, depth 4 (fetch -> prep -> front -> back),
              # with the back chain split into half-chunk stages and vlad(i)
              # emitted at the start of step i+1 so the PE runs the next
              # chunk's logits while the softmax chain of chunk i drains.
              for j in range(3):
                  if j < NL:
                      fetch(j)
              if NL > 0:
                  prep_sq(0)
                  prep_stats(0)
              if NL > 1:
                  prep_sq(1)
                  prep_stats(1)
              if NL > 0:
                  front_mm(0)
                  front_lsc(0, 0)
                  front_lsc(0, 1)
              for i in range(NL):
                  back_exp(i, 0)
                  if i > 0:
                      back_vlad(i - 1)
                  if i + 1 < NL:
                      front_mm(i + 1)
                  back_soft(i, 0)
                  back_a2(i, 0)
                  back_exp(i, 1)
                  if i + 2 < NL:
                      prep_sq(i + 2)
                  back_soft(i, 1)
                  back_a2(i, 1)
                  if i + 3 < NL:
                      fetch(i + 3)
                  if i + 2 < NL:
                      prep_stats(i + 2)
                  if i + 1 < NL:
                      front_lsc(i + 1, 0)
                      front_lsc(i + 1, 1)
              back_vlad(NL - 1)
            if reps > 1:
                with tc.For_i(0, reps, 1):
                    emit_all()
            else:
                emit_all()

    _split_waits(nc, mybir)
    return nc


_CACHE = {}


def _get_program(n_per_core, s_total, reps=1, n_read=None):
    key = (n_per_core, s_total, reps, n_read)
    if key not in _CACHE:
        _CACHE[key] = build_program(n_per_core, s_total, reps, n_read)
    return _CACHE[key]


def run_device(x, conv_w, conv_b, n_per_core=4, s_total=S, trace=False):
    """Run the device part. x: [NCORES*n_per_core, D, s_total] fp32.
    Returns (A [n, K, D], asum [n, K], bass_results)."""
    from concourse.bass_utils import run_bass_kernel_spmd

    nc = _get_program(n_per_core, s_total)

    bf16 = ml_dtypes.bfloat16
    f8 = ml_dtypes.float8_e4m3
    wt_np = np.ascontiguousarray(conv_w.T.astype(bf16))           # [D, K]
    idm_np = np.eye(D, dtype=bf16)                                 # [D, D]
    ones_np = np.ones((D, 1), bf16)
    bseed_np = np.concatenate(
        [conv_b.astype(np.float32), np.zeros(K, np.float32)]
    ).reshape(1, 2 * K).astype(f8)

    in_maps = []
    for c in range(NCORES):
        xc = np.ascontiguousarray(
            x[c * n_per_core : (c + 1) * n_per_core].astype(bf16)
        )
        in_maps.append(
            {"x": xc, "wt": wt_np, "idm": idm_np, "ones": ones_np,
             "bseed": bseed_np}
        )

    try:
        res = run_bass_kernel_spmd(
            nc, in_maps, list(range(NCORES)), trace=trace,
        )
    except Exception:
        # one retry: the device occasionally reports a transient
        # unrecoverable state right after a failed prior load
        time.sleep(2)
        res = run_bass_kernel_spmd(
            nc, in_maps, list(range(NCORES)), trace=trace,
        )

    n_total = NCORES * n_per_core
    A = np.empty((n_total, K, D), np.float64)
    asum = np.empty((n_total, K), np.float64)
    for c in range(NCORES):
        o = res.results[c]["out"]  # [K, n_per_core*132]
        for nl in range(n_per_core):
            blk = o[:, nl * 132 : nl * 132 + D + 1].astype(np.float64)
            A[c * n_per_core + nl] = blk[:, :D]
            asum[c * n_per_core + nl] = blk[:, D]
    return A, asum, res


def finalize(A, asum, centroids, att_w, att_b):
    cen = centroids.astype(np.float64)
    vlad = A - asum[:, :, None] * cen[None]
    soft = cen @ att_w.astype(np.float64).T + att_b.astype(np.float64)  # [K, 1]
    av = vlad * soft[None]
    nrm = np.maximum(np.linalg.norm(av, axis=2, keepdims=True), EPS)
    return (av / nrm).astype(np.float32)


def kernel(x, conv_w, conv_b, centroids, att_w, att_b):
    x = np.asarray(x, np.float32)
    A, asum, _ = run_device(
        x, np.asarray(conv_w, np.float32), np.asarray(conv_b, np.float32)
    )
    return finalize(
        A, asum,
        np.asarray(centroids, np.float32),
        np.asarray(att_w, np.float32),
        np.asarray(att_b, np.float32),
    )


# revision 21
# speedup vs baseline: 1.0184x; 1.0184x over previous
"""AttVlad Trainium2 kernel.

Math (per image n):
  xn = x / ||x||_2(over d)                       x: [D=128, S]
  a  = softmax_k(conv_w @ xn + conv_b)           a: [K=64, S]
  vlad[k,d]   = sum_s a[k,s] xn[d,s] - (sum_s a[k,s]) centroids[k,d]
  out = normalize_d(vlad * (centroids @ att_w.T + att_b))

Device strategy (8 cores, data-parallel over n, 4 images each), per
128-position unit u of each 4096-position chunk:
  - x is cast to bf16 on the host (device math is bf16 either way) and
    streamed in [128d, 4096s] chunks on the SP HWDGE queue.
  - PE per unit: (1) rank-1 fp8-DoubleRow seed writes norm[s]*b[k] into
    the logits PSUM, (2) bf16 logits matmul accumulates on top, so
    exp(rn*(l + norm*b)) = exp(rn*l + b) and the softmax bias multiply
    disappears; (3) an is_transpose matmul produces x^T in *bf16* PSUM.
  - lsc = psl*rn (DVE, broadcast); e = exp(lsc) (ACT, chunk-batched,
    k-major layout); dn = sum_k e (Pool/DVE); a2 = e * (rn*rdn) with the
    per-(s,u) factor materialized k-major by a 4x-mode tensor_copy so
    the a2 multiply runs in the DVE 2x bf16 mode.
  - x^T moves PSUM->SBUF as bf16 (DVE 2x tensor_copy / ACT copy / Pool,
    split per group) into a [128s, 130]-strided tile whose col 128 holds
    norm = sqrt(sumsq), so one 129-col VLAD matmul accumulates both
    A = sum_s a2 x^T and asum.
  - sumsq via natural-layout squares (ACT/DVE split) + per-unit ones
    matmul; rn = exp(-0.5 ln ss) stays in one ACT table set.
  - Host does the O(N*K*D) finalize (centroid subtract, attention scale,
    intra-normalization) in float64.
"""

import sys
import time

import numpy as np

try:  # the concourse stack (bass) ships in the container image
    import concourse.bass as _probe  # noqa: F401
except Exception:  # pragma: no cover
    sys.path.insert(0, "/opt/trn_rl_repo")

import ml_dtypes

N, D, S, K = 32, 128, 16384, 64
NCORES = 8
EPS = 1e-12

CHUNK = 4096  # s-positions per DMA chunk
UNIT = 128    # s-positions per matmul unit (psum partition dim)
XT_STRIDE = 130  # x^T unit stride in the SBUF tile: 128 cols x^T + 1 norm + 1 pad
UPC = CHUNK // UNIT  # units per chunk (32)
HU = 8               # units per pst group

# engine-split tunables (sweepable via the cost model)
XT_PATTERN = ("act", "act", "act", "act")   # xtmove engine per group g
DN_ENGINE = "dve"                            # dn reduce engine
A2_ENGINE = ("pool", "dve")                  # a2 engine per half
XSQ_DVE_COLS = 512                          # chunk cols squared on DVE (rest ACT)


# this walrus build rejects >1 sync wait on every instruction struct probed
# (CTRL, TT, MM); keep both caps at 1
MAX_WAITS = 1
COMPUTE_WAITS = 1
_COMPUTE_TYPES = (
    "InstTensorTensor", "InstActivation", "InstMatmult", "InstTensorReduce",
    "InstReciprocal", "InstTensorCopy", "InstLdweights", "InstTensorScalarPtr",
    "InstMemSet", "InstTensorScalar",
)


def _split_waits(nc, mybir):
    """Rewrite the traced BIR so no instruction carries more sem waits than
    this walrus build's per-struct limit: excess waits move to injected NoOps
    immediately preceding the instruction on the same engine (NX executes
    waits in order, so this is semantically identical)."""
    nid = 0
    for f in nc.m.functions:
        for blk in f.blocks:
            new_insts = []
            for inst in blk.instructions:
                si = getattr(inst, "sync_info", None)
                ws = list(si.on_wait) if si is not None else []
                maxw = (
                    COMPUTE_WAITS
                    if type(inst).__name__ in _COMPUTE_TYPES
                    else MAX_WAITS
                )
                if len(ws) > maxw:
                    extra = ws[: len(ws) - maxw]
                    for i in range(0, len(extra), MAX_WAITS):
                        nid += 1
                        nop = mybir.InstNoOp(
                            name=f"waitsplit_{nid}", ins=[], outs=[]
                        )
                        nop.engine = inst.engine
                        nop.sync_info = mybir.SyncInfo(
                            on_wait=extra[i : i + MAX_WAITS], on_update=[]
                        )
                        new_insts.append(nop)
                    si.on_wait = ws[len(ws) - maxw :]
                new_insts.append(inst)
            blk.instructions[:] = new_insts


def build_program(n_per_core=4, s_total=S, reps=1, n_read=None):
    """Build the single-core Bass program (same program runs on all cores)."""
    if n_read is None:
        n_read = n_per_core
    import concourse.bass as bass
    import concourse.tile as tile
    from concourse import mybir

    dt = mybir.dt
    AF = mybir.ActivationFunctionType
    OP = mybir.AluOpType
    PM = mybir.MatmulPerfMode

    n_chunks = s_total // CHUNK

    nc = bass.Bass()
    x_in = nc.declare_dram_parameter(
        "x", [n_per_core, D, s_total], dt.bfloat16, isOutput=False
    )
    wt_in = nc.declare_dram_parameter("wt", [D, K], dt.bfloat16, isOutput=False)
    idm_in = nc.declare_dram_parameter("idm", [D, D], dt.bfloat16, isOutput=False)
    ones_in = nc.declare_dram_parameter("ones", [D, 1], dt.bfloat16, isOutput=False)
    bseed_in = nc.declare_dram_parameter(
        "bseed", [1, 2 * K], dt.float8e4, isOutput=False
    )
    out_dram = nc.declare_dram_parameter(
        "out", [K, n_per_core * 132], dt.float32, isOutput=True
    )

    with tile.TileContext(nc) as tc:
        with (
            tc.tile_pool(name="consts", bufs=1) as consts,
            tc.tile_pool(name="xc", bufs=5) as xc_pool,
            tc.tile_pool(name="xt", bufs=6) as xt_pool,
            tc.tile_pool(name="soft", bufs=4) as soft_pool,
            tc.tile_pool(name="stats", bufs=5) as stats_pool,
            tc.tile_pool(name="scratch", bufs=2) as scratch_pool,
            tc.tile_pool(name="seed", bufs=3) as seed_pool,
            tc.tile_pool(name="outp", bufs=1) as out_pool,
            tc.tile_pool(name="psl", bufs=2, space="PSUM") as psl_pool,
            tc.tile_pool(name="pv", bufs=2, space="PSUM") as pv_pool,
            tc.tile_pool(name="pss", bufs=2, space="PSUM") as pss_pool,
        ):
            wt = consts.tile([D, K], dt.bfloat16)
            nc.sync.dma_start(wt[:], wt_in[:])
            idm = consts.tile([D, D], dt.bfloat16)
            nc.sync.dma_start(idm[:], idm_in[:])
            ones = consts.tile([D, 1], dt.bfloat16)
            nc.sync.dma_start(ones[:], ones_in[:])
            bseed = consts.tile([1, 2 * K], dt.float8e4)
            nc.sync.dma_start(bseed[:], bseed_in[:])
            bseed3 = bseed[:].rearrange("p (j n) -> p j n", j=2)

            out_sb = out_pool.tile([K, n_per_core * 132], dt.float32)
            # touch the ln/exp ACT table set immediately so its ~2.7us DMA
            # overlaps the initial input loads instead of the first chunk
            warm = consts.tile([1, 1], dt.float32)
            nc.scalar.activation(warm[:], ones[0:1, 0:1], AF.Ln)

            def emit_all():
              chunk_list = [
                  (n, ci) for n in range(n_read) for ci in range(n_chunks)
              ]
              NL = len(chunk_list)
              HC = UPC // 2  # units per half-chunk stage (16)
              st = {}
              pv_state = {}

              def fetch(i):
                  """Issue the chunk load and the XBAR transpose (DMA)."""
                  n, ci = chunk_list[i]
                  xc = xc_pool.tile([D, CHUNK], dt.bfloat16, name="xc")
                  QC = CHUNK // 4
                  if i == 0:
                      for q in range(4):
                          nc.sync.dma_start(
                              xc[:, q * QC : (q + 1) * QC],
                              x_in[n, :, ci * CHUNK + q * QC
                                   : ci * CHUNK + (q + 1) * QC],
                          )
                  else:
                      nc.sync.dma_start(
                          xc[:], x_in[n, :, ci * CHUNK : (ci + 1) * CHUNK]
                      )
                  # x^T via the XBAR DMA transpose (u-major: xt3[p,u,:] =
                  # x^T[u*128+p]); rides the otherwise-idle DMA rails
                  xt = xt_pool.tile([128, UPC * UNIT], dt.bfloat16,
                                    name="xt")
                  xt3 = xt[:].rearrange("p (u c) -> p u c", c=UNIT)
                  nc.sync.dma_start_transpose(xt3, xc[:])
                  st[i] = {"xc": xc, "xt": xt}

              def prep_sq(i):
                  """Squares for chunk i (ACT, with a DVE share)."""
                  t = st[i]
                  xc = t["xc"]
                  xsq = scratch_pool.tile(
                      [D, CHUNK], dt.bfloat16, tag="xsq", name="xsq"
                  )
                  cd = XSQ_DVE_COLS
                  if i == 0:
                      QC = CHUNK // 4
                      for q in range(4):
                          nc.vector.tensor_tensor(
                              out=xsq[:, q * QC : (q + 1) * QC],
                              in0=xc[:, q * QC : (q + 1) * QC],
                              in1=xc[:, q * QC : (q + 1) * QC], op=OP.mult,
                          )
                  else:
                      if cd > 0:
                          nc.vector.tensor_tensor(
                              out=xsq[:, 0:cd], in0=xc[:, 0:cd],
                              in1=xc[:, 0:cd], op=OP.mult,
                          )
                      if cd < CHUNK:
                          nc.scalar.activation(
                              xsq[:, cd:CHUNK], xc[:, cd:CHUNK], AF.Square
                          )
                  t["xsq"] = xsq

              def prep_stats(i):
                  """sumsq matmuls, rn, norm, fp8 seed row for chunk i."""
                  t = st[i]
                  xsq = t.pop("xsq")
                  # pss bank: ss [128, 0:32] fp32; seedT [32, 32:96]->bf16
                  pss = pss_pool.tile([128, 96], dt.float32, name="pss")
                  ss = pss[:, 0:32]
                  for cu in range(UPC):
                      nc.tensor.matmul(
                          ss[:, cu : cu + 1],
                          xsq[:, cu * UNIT : (cu + 1) * UNIT],
                          ones[:], start=True, stop=True,
                      )
                  lns = stats_pool.tile([128, UPC], dt.float32, tag="lns",
                                        name="lns")
                  rn = stats_pool.tile([128, UPC], dt.float32, tag="rn",
                                       name="rn")
                  nc.scalar.activation(lns[:], ss, AF.Ln)
                  nc.scalar.activation(rn[:], lns[:], AF.Exp, scale=-0.5)
                  # norm = sqrt(ss), compact bf16 (seed row + asum column)
                  nr16 = stats_pool.tile([128, UPC], dt.bfloat16, tag="nr16",
                                         name="nr16")
                  nc.scalar.activation(nr16[:], lns[:], AF.Exp, scale=0.5)
                  # seed row: transpose norms to [32s-units, 128] then fp8
                  seedT = pss[0:32, 32:96].bitcast(dt.bfloat16)
                  nc.tensor.transpose(seedT, nr16[:], idm[:])
                  seed8 = seed_pool.tile([UPC, UNIT], dt.float8e4,
                                         tag="s8", name="seed8")
                  nc.scalar.activation(seed8[:], seedT, AF.Copy)
                  # partition-collapse: PE stationary base partition must be
                  # 0/32/64, so gather all unit rows onto partition 0
                  seed8f = seed_pool.tile([1, UPC * UNIT], dt.float8e4,
                                          tag="s8f", name="seed8f")
                  nc.sync.dma_start(seed8f[:], seed8[:])
                  t.update(rn=rn, nr16=nr16, seed8f=seed8f)

              def front_mm(i):
                  """Seed + logits matmuls for chunk i (both halves)."""
                  t = st[i]
                  xc, seed8f = t["xc"], t.pop("seed8f")
                  psls = []
                  for h in range(2):
                      psl = psl_pool.tile([128, HC * K], dt.float32,
                                          name="psl")
                      for ul in range(HC):
                          u = h * HC + ul
                          srow = seed8f[0:1, u * UNIT : (u + 1) * UNIT][
                              :, None, :
                          ]
                          nc.tensor.matmul(
                              psl[:, ul * K : (ul + 1) * K],
                              srow.broadcast_to([1, 2, UNIT]),
                              bseed3,
                              start=True, stop=False, perf_mode=PM.DoubleRow,
                          )
                          nc.tensor.matmul(
                              psl[:, ul * K : (ul + 1) * K],
                              xc[:, u * UNIT : (u + 1) * UNIT], wt[:],
                              start=False, stop=True,
                          )
                      psls.append(psl)
                  t["psls"] = psls

              def front_lsc(i, h):
                  """lsc_h = psl_h * rn_h, k-major [p, (k,64),(u,16)]."""
                  t = st[i]
                  psl = t["psls"][h]
                  rn = t["rn"]
                  lsc = soft_pool.tile([128, HC * K], dt.bfloat16, tag="lsc",
                                       name="lsc")
                  lsc_v = lsc[:].rearrange("p (k u) -> p k u", u=HC)
                  psl_v = psl[:].rearrange("p (u k) -> p k u", k=K)
                  rn_v = rn[:, h * HC : (h + 1) * HC][:, None, :]
                  nc.vector.tensor_tensor(
                      out=lsc_v, in0=psl_v,
                      in1=rn_v.broadcast_to([128, K, HC]), op=OP.mult,
                  )
                  t[("lsc", h)] = lsc

              def back_exp(i, h):
                  t = st[i]
                  lsc = t.pop(("lsc", h))
                  et = soft_pool.tile([128, HC * K], dt.bfloat16, tag="e",
                                      name="e")
                  nc.scalar.activation(et[:], lsc[:], AF.Exp)
                  t[("et", h)] = et

              def back_soft(i, h):
                  """dn_h + per-(s,u) scalar chain + ccb_h (DVE)."""
                  t = st[i]
                  et = t[("et", h)]
                  rn = t["rn"]
                  dn = stats_pool.tile([128, HC], dt.float32, tag="dn",
                                       name="dn")
                  e_ku = et[:].rearrange("p (k u) -> p u k", u=HC)
                  nc.vector.tensor_reduce(
                      out=dn[:], in_=e_ku, axis=mybir.AxisListType.X,
                      op=OP.add,
                  )
                  rdn = stats_pool.tile([128, HC], dt.float32, tag="rdn",
                                        name="rdn")
                  nc.vector.reciprocal(rdn[:], dn[:])
                  cc = stats_pool.tile([128, HC], dt.float32, tag="cc",
                                       name="cc")
                  nc.vector.tensor_tensor(
                      out=cc[:], in0=rn[:, h * HC : (h + 1) * HC],
                      in1=rdn[:], op=OP.mult,
                  )
                  cc16 = stats_pool.tile([128, HC], dt.bfloat16, tag="cc16",
                                         name="cc16")
                  nc.vector.tensor_copy(cc16[:], cc[:])
                  ccb = soft_pool.tile([128, HC * K], dt.bfloat16, tag="ccb",
                                       name="ccb")
                  nc.vector.tensor_copy(
                      ccb[:].rearrange("p (k u) -> p k u", u=HC),
                      cc16[:][:, None, :].broadcast_to([128, K, HC]),
                  )
                  t[("ccb", h)] = ccb

              def back_a2(i, h):
                  t = st[i]
                  et, ccb = t.pop(("et", h)), t.pop(("ccb", h))
                  a2 = soft_pool.tile([128, HC * K], dt.bfloat16, tag="a2",
                                      name="a2")
                  if A2_ENGINE[h] == "pool":
                      nc.gpsimd.tensor_tensor(out=a2[:], in0=et[:],
                                              in1=ccb[:], op=OP.mult)
                  else:
                      nc.vector.tensor_tensor(out=a2[:], in0=et[:],
                                              in1=ccb[:], op=OP.mult)
                  t[("a2", h)] = a2

              def back_vlad(i):
                  """VLAD + asum accumulation; output on the last chunk."""
                  n, ci = chunk_list[i]
                  t = st.pop(i)
                  xt, nr16 = t["xt"], t["nr16"]
                  if ci == 0:
                      pv_state[n] = pv_pool.tile([K, 132], dt.float32,
                                                 name="pv")
                  pv = pv_state[n]
                  for h in range(2):
                      a2 = t[("a2", h)]
                      a2_ku = a2[:].rearrange("p (k u) -> p u k", u=HC)
                      for ul in range(HC):
                          u = h * HC + ul
                          cu = ci * UPC + u
                          first = cu == 0
                          last = cu == (s_total // UNIT) - 1
                          nc.tensor.matmul(
                              pv[:, 0:D],
                              a2_ku[:, ul, :],
                              xt[:, u * UNIT : (u + 1) * UNIT],
                              start=first, stop=last,
                          )
                          nc.tensor.matmul(
                              pv[:, D : D + 1],
                              a2_ku[:, ul, :],
                              nr16[:, u : u + 1],
                              start=first, stop=last,
                          )
                  if ci == n_chunks - 1:
                      nc.scalar.activation(
                          out_sb[:, n * 132 : n * 132 + D + 1],
                          pv[:, 0 : D + 1], AF.Copy,
                      )
                      nc.sync.dma_start(
                          out_dram[:, n * 132 : n * 132 + D + 1],
                          out_sb[:, n * 132 : n * 132 + D + 1],
                      )

              # Software pipeline, depth 4 (fetch -> prep -> front -> back),
              # with the back chain split into half-chunk stages and vlad(i)
              # emitted at the start of step i+1 so the PE runs the next
              # chunk's logits while the softmax chain of chunk i drains.
              for j in range(3):
                  if j < NL:
                      fetch(j)
              if NL > 0:
                  prep_sq(0)
                  prep_stats(0)
              if NL > 1:
                  prep_sq(1)
                  prep_stats(1)
              if NL > 0:
                  front_mm(0)
                  front_lsc(0, 0)
                  front_lsc(0, 1)
              for i in range(NL):
                  back_exp(i, 0)
                  if i + 1 < NL:
                      front_mm(i + 1)
                  back_soft(i, 0)
                  back_a2(i, 0)
                  back_exp(i, 1)
                  if i + 2 < NL:
                      prep_sq(i + 2)
                  back_soft(i, 1)
                  back_a2(i, 1)
                  if i + 3 < NL:
                      fetch(i + 3)
                  if i + 2 < NL:
                      prep_stats(i + 2)
                  if i + 1 < NL:
                      front_lsc(i + 1, 0)
                      front_lsc(i + 1, 1)
                  if i > 0:
                      back_vlad(i - 1)
              back_vlad(NL - 1)
            if reps > 1:
                with tc.For_i(0, reps, 1):
                    emit_all()
            else:
                emit_all()

    _split_waits(nc, mybir)
    return nc


_CACHE = {}


def _get_program(n_per_core, s_total, reps=1, n_read=None):
    key = (n_per_core, s_total, reps, n_read)
    if key not in _CACHE:
        _CACHE[key] = build_program(n_per_core, s_total, reps, n_read)
    return _CACHE[key]


def run_device(x, conv_w, conv_b, n_per_core=4, s_total=S, trace=False):
    """Run the device part. x: [NCORES*n_per_core, D, s_total] fp32.
    Returns (A [n, K, D], asum [n, K], bass_results)."""
    from concourse.bass_utils import run_bass_kernel_spmd

    nc = _get_program(n_per_core, s_total)

    bf16 = ml_dtypes.bfloat16
    f8 = ml_dtypes.float8_e4m3
    wt_np = np.ascontiguousarray(conv_w.T.astype(bf16))           # [D, K]
    idm_np = np.eye(D, dtype=bf16)                                 # [D, D]
    ones_np = np.ones((D, 1), bf16)
    bseed_np = np.concatenate(
        [conv_b.astype(np.float32), np.zeros(K, np.float32)]
    ).reshape(1, 2 * K).astype(f8)

    in_maps = []
    for c in range(NCORES):
        xc = np.ascontiguousarray(
            x[c * n_per_core : (c + 1) * n_per_core].astype(bf16)
        )
        in_maps.append(
            {"x": xc, "wt": wt_np, "idm": idm_np, "ones": ones_np,
             "bseed": bseed_np}
        )

    try:
        res = run_bass_kernel_spmd(
            nc, in_maps, list(range(NCORES)), trace=trace,
        )
    except Exception:
        # one retry: the device occasionally reports a transient
        # unrecoverable state right after a failed prior load
        time.sleep(2)
        res = run_bass_kernel_spmd(
            nc, in_maps, list(range(NCORES)), trace=trace,
        )

    n_total = NCORES * n_per_core
    A = np.empty((n_total, K, D), np.float64)
    asum = np.empty((n_total, K), np.float64)
    for c in range(NCORES):
        o = res.results[c]["out"]  # [K, n_per_core*132]
        for nl in range(n_per_core):
            blk = o[:, nl * 132 : nl * 132 + D + 1].astype(np.float64)
            A[c * n_per_core + nl] = blk[:, :D]
            asum[c * n_per_core + nl] = blk[:, D]
    return A, asum, res


def finalize(A, asum, centroids, att_w, att_b):
    cen = centroids.astype(np.float64)
    vlad = A - asum[:, :, None] * cen[None]
    soft = cen @ att_w.astype(np.float64).T + att_b.astype(np.float64)  # [K, 1]
    av = vlad * soft[None]
    nrm = np.maximum(np.linalg.norm(av, axis=2, keepdims=True), EPS)
    return (av / nrm).astype(np.float32)


def kernel(x, conv_w, conv_b, centroids, att_w, att_b):
    x = np.asarray(x, np.float32)
    A, asum, _ = run_device(
        x, np.asarray(conv_w, np.float32), np.asarray(conv_b, np.float32)
    )
    return finalize(
        A, asum,
        np.asarray(centroids, np.float32),
        np.asarray(att_w, np.float32),
        np.asarray(att_b, np.float32),
    )
